# revision 24
# baseline (speedup 1.0000x reference)
"""XCA-style attention block (qkv 1x1 conv -> depthwise 3x3 -> L2-normed
cross-covariance attention -> 1x1 proj) on 8 TRN2 NeuronCores.

Sharding: core i handles (batch b = i//2, image half hf = i%2): 128 rows of
the 256-row image. Per-pair [128,195] stats all-reduce; softmax + temperature
+ norm fixups fold into W2 = proj @ blockdiag(attn); phase 2 = W2 @ v.

Phase-1 redesign vs the old kernel: 32 chunks x 4 output rows, each
self-contained (qkv recomputed for the 2 halo rows). The 3x3 center tap
(1,1) is folded into the qkv weights (psum = w11*qkv; raw qkv recovered by
the ACT evacuation with scale=1/w11 into a row-padded "ring" buffer: 260
stride, zero gap cols => no wrap patches, no shifted qB copy). Five side
taps run as per-row N=256 diagonal matmuls accumulating onto the same psum.
Tap (2,2) folds into the DVE scalar_tensor_tensor psum evacuation; taps
(0,1),(2,1) are aligned DVE tensor_scalar+tensor_tensor pairs. v channels
384..511 stay resident in SBUF for phase 2; 512..575 spill to DRAM.
"""
import numpy as np
import ml_dtypes

import concourse.bass as bass
import concourse.tile as tile
from concourse import mybir
from concourse.bass_utils import run_bass_kernel_spmd
from concourse.masks import make_identity

# --- patch: this walrus build rejects >1 semaphore wait on a Drain ---------
import concourse.tile as _tile_mod
from concourse.vector_clock import ScopedClock as _SC, VectorClock as _VC


def _drain_and_barrier(self, tick_clock, wait_clock):
    gc = tick_clock.global_clock
    n = len(gc)
    nonzero = [i for i in range(n) if gc[i] > 0]
    for i in nonzero:
        vec = [gc[j] if j == i else 0 for j in range(n)]
        inst = self.nc.sync.drain()
        wait_clock.add_sem_waits(inst.ins, _SC({None: _VC(vec)}))
    if not nonzero:
        inst = self.nc.sync.drain()
        wait_clock.add_sem_waits(inst.ins, _SC({None: gc}))
    self.nc.all_engine_barrier()
    assert self.sems is not None
    popped = self.nc._tile_sem_poison_stack.pop()
    assert popped is self._sem_poison
    self.nc.clear_and_free_semaphores(list(self.sems.allocated().values()))
    self.nc.all_engine_barrier()


_tile_mod.TileContext._drain_and_barrier = _drain_and_barrier

# The same walrus limit applies to every engine instruction: at most ONE
# semaphore wait. Split extra waits onto preceding same-engine NoOps (engines
# execute in order, so earlier waits still gate the instruction). DMA copies
# use the descriptor path and tolerate multiple waits, so leave them alone.
_orig_commit_and_lower = _tile_mod.TileContext._commit_and_lower
_split_counter = [0]


def _commit_and_lower_split(self, inst, original_block, old_bb_map, bb_to_exit_bb):
    si = getattr(inst, "sync_info", None)
    if si is not None and len(si.on_wait) > 1 and inst.engine is not None:
        waits = list(si.on_wait)
        for w in waits[:-1]:
            _split_counter[0] += 1
            nop = mybir.InstNoOp(
                name=f"{inst.name}-wsplit{_split_counter[0]}",
                sync_info=mybir.SyncInfo(on_wait=[w], on_update=[]),
                bass_nofuse=True,
                engine=inst.engine,
            )
            self._commit_instruction(nop)
        inst.sync_info = mybir.SyncInfo(on_wait=[waits[-1]], on_update=list(si.on_update))
    return _orig_commit_and_lower(self, inst, original_block, old_bb_map, bb_to_exit_bb)


_tile_mod.TileContext._commit_and_lower = _commit_and_lower_split
# ---------------------------------------------------------------------------

F32 = mybir.dt.float32
BF16 = mybir.dt.bfloat16
AX = mybir.AxisListType
OP = mybir.AluOpType
ACTF = mybir.ActivationFunctionType

B, C, H, W = 4, 192, 256, 256
HEADS, HD = 8, 24
C3 = 3 * C  # 576
HALF = H // 2  # rows per core
CH = 4  # output rows per chunk
NCH = HALF // CH  # 32
RIN = CH + 2  # rows incl halo
SW = W + 4  # padded ring row stride (260)
PXC = CH * W  # 1024 px per chunk
CT = [128, 128, 128, 128, 64]
CTO = [0, 128, 256, 384, 512]
NIT = NCH * 5  # flattened (chunk, ct) iterations
PAIRS = [[0, 1], [2, 3], [4, 5], [6, 7]]
CCSPLIT = 26  # chunks [0, CCSPLIT) all-reduce early, hidden behind the tail

# PE side taps (dy, dx); (1,1) folded into qkv weights, (2,2) folded into
# the DVE stt evacuation, (0,1)/(2,1) are DVE ts+tt pairs.
PE_TAPS = [(0, 0), (0, 2), (1, 0), (1, 2), (2, 0)]


def build_nc():
    nc = bass.Bass()
    x_ext = nc.declare_dram_parameter("xin", [C, (HALF + 2) * W], BF16, isOutput=False)
    qkvwt_ext = nc.declare_dram_parameter("qkvwt", [C, C3], BF16, isOutput=False)
    projt_ext = nc.declare_dram_parameter("projt", [C, C], BF16, isOutput=False)
    dwd_ext = nc.declare_dram_parameter("dwdiag", [len(PE_TAPS) * 128, C3], BF16, isOutput=False)
    dwpair_ext = nc.declare_dram_parameter("dwpair", [2 * 128, 64], BF16, isOutput=False)
    dvecol_ext = nc.declare_dram_parameter("dvecol", [128, 15], F32, isOutput=False)
    recip_ext = nc.declare_dram_parameter("recip11", [128, 5], F32, isOutput=False)
    tempcol_ext = nc.declare_dram_parameter("tempcol", [128, 2], F32, isOutput=False)
    out_ext = nc.declare_dram_parameter("out", [C, HALF * W], BF16, isOutput=True)

    with tile.TileContext(nc) as tc:
        with tc.tile_pool(name="wpool", bufs=1) as wp, \
             tc.tile_pool(name="dram", bufs=1, space="DRAM") as dram:
            # ---- weights / constants
            qkvw0 = wp.tile([128, C3], BF16)
            qkvw1 = wp.tile([128, C3], BF16)
            nc.sync.dma_start(out=qkvw0[:], in_=qkvwt_ext[0:128, :])
            nc.sync.dma_start(out=qkvw1[0:64, :], in_=qkvwt_ext[128:192, :])
            nc.sync.dma_start(out=qkvw1[64:128, :], in_=qkvwt_ext[128:192, :])
            projt0 = wp.tile([128, C], BF16)
            projt1 = wp.tile([64, C], BF16)
            nc.sync.dma_start(out=projt0[:], in_=projt_ext[0:128, :])
            nc.sync.dma_start(out=projt1[:], in_=projt_ext[128:192, :])
            dwd = [wp.tile([128, C3], BF16, name=f"dwd{s}") for s in range(len(PE_TAPS))]
            for s in range(len(PE_TAPS)):
                nc.sync.dma_start(out=dwd[s][:], in_=dwd_ext[128 * s:128 * (s + 1), :])
            dwpair = [wp.tile([128, 64], BF16, name=f"dwp{g}") for g in range(2)]
            for g in range(2):
                nc.sync.dma_start(out=dwpair[g][:], in_=dwpair_ext[128 * g:128 * (g + 1), :])
            dvecol = wp.tile([128, 15], F32)
            nc.sync.dma_start(out=dvecol[:], in_=dvecol_ext[:])
            recip11 = wp.tile([128, 5], F32)
            nc.sync.dma_start(out=recip11[:], in_=recip_ext[:])
            tempcol = wp.tile([128, 2], F32)
            nc.sync.dma_start(out=tempcol[:], in_=tempcol_ext[:])
            ident = wp.tile([128, 128], BF16)
            make_identity(nc, ident[:])
            ident32 = wp.tile([128, 128], F32)
            make_identity(nc, ident32[:])
            ones32 = wp.tile([1, 128], F32)
            nc.vector.memset(ones32[:], 1.0)

            # persistent accumulators / residents
            sq_part = [wp.tile([CT[ct], NCH], F32, name=f"sqp{ct}") for ct in range(3)]
            v3res = wp.tile([128, HALF * W], BF16)  # v channels 384..511
            v4_dram = dram.tile([64, HALF * W], BF16)
            stats = wp.tile([128, 195], F32)
            stats2 = wp.tile([128, 195], F32)
            statf1 = wp.tile([128, 195], F32)
            cc_in = dram.tile([128, 195], F32)
            cc_out = dram.tile([128, 195], F32)
            cc_in2 = dram.tile([128, 195], F32)
            cc_out2 = dram.tile([128, 195], F32)

            def pe_warm(n, tag):
                # dummy matmuls keeping the PE HAM busy across PE-idle spans
                # (DMA prologue/collective/epilogue) so real matmuls run at 2.4 GHz
                with tc.tile_pool(name=f"wm{tag}", bufs=1, space="PSUM") as wpp:
                    wps = wpp.tile([128, 128], F32)
                    for _ in range(n):
                        nc.tensor.matmul(wps[:], ident[:], ident[:],
                                         start=True, stop=True)

            pe_warm(350, "start")

            # ================= phase 1 =================
            with tc.tile_pool(name="p1", bufs=2) as p1, \
                 tc.tile_pool(name="ps1", bufs=2, space="PSUM") as ps1, \
                 tc.tile_pool(name="gps", bufs=1, space="PSUM") as gps:
                gAB = gps.tile([96, 192], F32, tag="gAB")
                gA = gAB[:, 0:96]
                gB = gAB[:, 96:192]

                state = {}  # per live iteration: ring view, psum, etc.

                def issue_qkv(it):
                    c, ct = divmod(it, 5)
                    cw = CT[ct]
                    cts = slice(CTO[ct], CTO[ct] + cw)
                    r0 = 4 * c  # first xin row of the 6-row window
                    # x tiles for this chunk are shared across cts via tags
                    if ct == 0:
                        x0 = p1.tile([128, RIN * W], BF16, tag="x0", bufs=3)
                        x1 = p1.tile([128, RIN * W], BF16, tag="x1", bufs=3)
                        nc.sync.dma_start(out=x0[:], in_=x_ext[0:128, r0 * W:(r0 + RIN) * W])
                        nc.sync.dma_start(out=x1[0:64, :], in_=x_ext[128:192, r0 * W:(r0 + RIN) * W])
                        nc.sync.dma_start(out=x1[64:128, :], in_=x_ext[128:192, r0 * W:(r0 + RIN) * W])
                        state[("x", c)] = (x0, x1)
                    x0, x1 = state[("x", c)]

                    Bm = ps1.tile([cw, 4 * W], F32, tag="Bm", name=f"Bm{ct}")
                    Bh = ps1.tile([cw, 2 * W], F32, tag="Bh", bufs=1, name=f"Bh{ct}")
                    # main rows (xin rows r0+1..r0+4): two 512 blocks
                    nc.tensor.matmul(Bm[:, 0:512], qkvw0[:, cts], x0[:, W:3 * W],
                                     start=True, stop=False)
                    nc.tensor.matmul(Bm[:, 512:1024], qkvw0[:, cts], x0[:, 3 * W:5 * W],
                                     start=True, stop=False)
                    nc.tensor.matmul(Bm[:, 0:512], qkvw1[0:64, cts], x1[0:64, W:3 * W],
                                     start=False, stop=False, tile_position=(0, 0),
                                     skip_group_check=True)
                    nc.tensor.matmul(Bm[:, 512:1024], qkvw1[64:128, cts], x1[64:128, 3 * W:5 * W],
                                     start=False, stop=False, tile_position=(64, 0),
                                     skip_group_check=True)
                    x0v = x0[:].rearrange("p (r w) -> p r w", w=W)
                    x1v = x1[:].rearrange("p (r w) -> p r w", w=W)
                    if c == 0:
                        # halo rows r0, r0+5 -> Bh (strided 2-row rhs view)
                        nc.tensor.matmul(Bh[:], qkvw0[:, cts], x0v[:, 0:6:5, :],
                                         start=True, stop=False)
                        nc.tensor.matmul(Bh[:], qkvw1[0:64, cts], x1v[0:64, 0:6:5, :],
                                         start=False, stop=True, tile_position=(0, 0),
                                         skip_group_check=True)
                    else:
                        # only row r0+5; row r0 is copied from the previous ring
                        nc.tensor.matmul(Bh[:, 256:512], qkvw0[:, cts], x0v[:, 5, :],
                                         start=True, stop=False)
                        nc.tensor.matmul(Bh[:, 256:512], qkvw1[0:64, cts], x1v[0:64, 5, :],
                                         start=False, stop=True, tile_position=(0, 0),
                                         skip_group_check=True)

                    # ring: padded raw-qkv rows (scale 1/w11). ct4 has a +2-col
                    # shifted duplicate on partitions 64:128 for tap pairing.
                    prows = 128 if ct == 4 else cw
                    ring = p1.tile([prows, RIN * SW], BF16, tag=f"ring{ct}", name=f"ring{ct}")
                    if c < 2:
                        nc.gpsimd.memset(ring[:], 0.0)
                    rv = ring[0:cw].rearrange("p (r w) -> p r w", w=SW)
                    rsc = recip11[0:cw, ct:ct + 1]
                    Bmv = Bm[:].rearrange("p (r w) -> p r w", w=W)
                    nc.scalar.activation(out=rv[:, 1:5, 2:2 + W], in_=Bmv,
                                         func=ACTF.Copy, scale=rsc)
                    if c == 0:
                        Bhv = Bh[:].rearrange("p (r w) -> p r w", w=W)
                        nc.scalar.activation(out=rv[:, 0:6:5, 2:2 + W], in_=Bhv,
                                             func=ACTF.Copy, scale=rsc)
                    else:
                        nc.scalar.activation(out=rv[:, 5:6, 2:2 + W],
                                             in_=Bh[:, 256:512],
                                             func=ACTF.Copy, scale=rsc)
                        rvp = state[("rv", c - 1, ct)]
                        nc.sync.dma_start(out=rv[:, 0, 2:2 + W], in_=rvp[:, 4, 2:2 + W])
                    state[("rv", c, ct)] = rv
                    state.pop(("rv", c - 2, ct), None)
                    if ct == 4:
                        # dup partitions 64:128 = data shifted +2 cols
                        r4f = ring[:].rearrange("p (r w) -> p r w", w=SW)
                        nc.sync.dma_start(out=r4f[64:128, :, 0:SW - 2],
                                          in_=r4f[0:64, :, 2:SW])
                        state[("r4", it)] = r4f
                    state[("B", it)] = (Bm, ring, rv)

                def issue_consume(it):
                    c, ct = divmod(it, 5)
                    cw = CT[ct]
                    cts = slice(CTO[ct], CTO[ct] + cw)
                    Bm, ring, rv = state.pop(("B", it))

                    # 5 side taps: per-output-row N=256 diag MMs, tap-outer.
                    # ct4 packs ((0,0),(0,2)) and ((1,0),(1,2)) via the +2-col
                    # dup on partitions 64:128 (one 128-contraction MM each).
                    if ct == 4:
                        r4f = state.pop(("r4", it))
                        for g, dy in ((0, 0), (1, 1)):
                            for j in range(4):
                                nc.tensor.matmul(
                                    Bm[:, j * W:(j + 1) * W], dwpair[g][:],
                                    r4f[:, j + dy, 1:1 + W],
                                    start=False, stop=False,
                                    skip_group_check=True)
                        for j in range(4):
                            nc.tensor.matmul(
                                Bm[:, j * W:(j + 1) * W], dwd[4][0:cw, cts],
                                rv[:, j + 2, 1:1 + W],
                                start=False, stop=True,
                                skip_group_check=True)
                    else:
                        for s, (dy, dx) in enumerate(PE_TAPS):
                            lw = dwd[s][0:cw, cts]
                            for j in range(4):
                                nc.tensor.matmul(
                                    Bm[:, j * W:(j + 1) * W], lw,
                                    rv[:, j + dy, 1 + dx:1 + dx + W],
                                    start=False, stop=(s == len(PE_TAPS) - 1),
                                    skip_group_check=True)

                    # evacuate + fold tap (2,2): acc = ring(2,2)*w22 + psum
                    if ct == 3:
                        acc = v3res[:, c * PXC:(c + 1) * PXC]
                    else:
                        acc_t = p1.tile([cw, PXC], BF16, tag=f"acc{ct}", name=f"acc{ct}")
                        acc = acc_t[:]
                    nc.vector.scalar_tensor_tensor(
                        out=acc, in0=rv[:, 2:6, 3:3 + W],
                        scalar=dvecol[0:cw, 3 * ct + 2:3 * ct + 3],
                        in1=Bm[:], op0=OP.mult, op1=OP.add)
                    # taps (0,1) and (2,1): ts mult (DVE) + tt add (DVE / GpSimd)
                    for k, dy in ((0, 0), (1, 2)):
                        tmp = p1.tile([cw, PXC], BF16, tag="dwtmp", name="dwtmp")
                        nc.vector.tensor_scalar(
                            out=tmp[:], in0=rv[:, dy:dy + 4, 2:2 + W],
                            scalar1=dvecol[0:cw, 3 * ct + k:3 * ct + k + 1],
                            scalar2=None, op0=OP.mult)
                        if k == 1 and c < CCSPLIT:
                            nc.gpsimd.tensor_tensor(out=acc, in0=acc, in1=tmp[:], op=OP.add)
                        else:
                            nc.vector.tensor_tensor(out=acc, in0=acc, in1=tmp[:], op=OP.add)

                    if ct < 3:
                        scr = p1.tile([cw, PXC], BF16, tag="sqscr")
                        nc.scalar.activation(out=scr[:], in_=acc, func=ACTF.Square,
                                             accum_out=sq_part[ct][:, c:c + 1])
                        state[("acc", c, ct)] = acc
                    if ct == 4:
                        nc.sync.dma_start(out=v4_dram[:, c * PXC:(c + 1) * PXC], in_=acc)

                def issue_gram(c):
                    accs = [state.pop(("acc", c, ct)) for ct in range(3)]
                    # 8 px-blocks of 128; 2 pb per tg/qkt tile
                    for g in range(4):
                        tg = gps.tile([128, 768], BF16, tag="tg", bufs=2)
                        qkt = p1.tile([128, 768], BF16, tag="qkt", bufs=3)
                        for h in range(2):
                            pb = 2 * g + h
                            pbs = slice(pb * 128, (pb + 1) * 128)
                            for q in range(3):
                                nc.tensor.transpose(
                                    tg[:, h * 384 + q * 128:h * 384 + (q + 1) * 128],
                                    accs[q][:, pbs], ident[:])
                        nc.scalar.copy(qkt[:], tg[:])
                        for h in range(2):
                            o = h * 384
                            first = (c in (0, CCSPLIT) and g == 0 and h == 0)
                            last = (c in (CCSPLIT - 1, NCH - 1) and g == 3 and h == 1)
                            nc.tensor.matmul(gA, qkt[:, o:o + 96], qkt[:, o + 192:o + 288],
                                             start=first, stop=last)
                            nc.tensor.matmul(gB, qkt[:, o + 96:o + 192], qkt[:, o + 288:o + 384],
                                             start=first, stop=last)

                for it in range(NIT + 1):
                    if it < NIT:
                        issue_qkv(it)
                    if it >= 1:
                        issue_consume(it - 1)
                        c, ct = divmod(it - 1, 5)
                        if ct == 3:
                            issue_gram(c)
                        if ct == 3 and c == CCSPLIT - 1:
                            # part-A stats all-reduce, hidden behind the
                            # remaining chunks
                            for q in range(3):
                                nc.vector.tensor_reduce(
                                    out=stats[0:CT[q], 192 + q:193 + q],
                                    in_=sq_part[q][:, 0:CCSPLIT], axis=AX.X, op=OP.add)
                            nc.scalar.copy(stats[0:96, 0:192], gAB[:])
                            nc.sync.dma_start(out=cc_in[:], in_=stats[:])
                            nc.gpsimd.collective_compute(
                                "AllReduce", OP.add, replica_groups=PAIRS,
                                ins=[cc_in.opt()], outs=[cc_out.opt()])
                            nc.sync.dma_start(out=statf1[:], in_=cc_out[:])

                # part-B stats; pack stats2 = [gA | gB | ssq]
                for ct in range(3):
                    nc.vector.tensor_reduce(
                        out=stats2[0:CT[ct], 192 + ct:193 + ct],
                        in_=sq_part[ct][:, CCSPLIT:NCH], axis=AX.X, op=OP.add)
                nc.scalar.copy(stats2[0:96, 0:192], gAB[:])

            # ================= collective (part B) =================
            nc.sync.dma_start(out=cc_in2[:], in_=stats2[:])
            nc.gpsimd.collective_compute(
                "AllReduce", OP.add, replica_groups=PAIRS,
                ins=[cc_in2.opt()], outs=[cc_out2.opt()])
            statf = wp.tile([128, 195], F32)
            nc.sync.dma_start(out=statf[:], in_=cc_out2[:])
            nc.vector.tensor_tensor(out=statf[:], in0=statf[:], in1=statf1[:], op=OP.add)
            pe_warm(400, "cc")

            # ================= epilogue (tiny) =================
            with tc.tile_pool(name="ep", bufs=1) as ep:
              with tc.tile_pool(name="eps", bufs=1, space="PSUM") as eps:
                # 1/max(sqrt(ssq), eps) per q/k channel, [128, 3] by ct
                nrm = ep.tile([128, 3], F32)
                nc.scalar.activation(out=nrm[:], in_=statf[:, 192:195], func=ACTF.Sqrt)
                nc.vector.tensor_scalar(out=nrm[:], in0=nrm[:], scalar1=1e-12,
                                        scalar2=None, op0=OP.max)
                rn = ep.tile([128, 3], F32)
                nc.vector.reciprocal(rn[:], nrm[:])

                # row scales (q-norms * temperature), partition-packed per gram tile
                rsA = ep.tile([96, 1], F32)
                nc.vector.tensor_tensor(out=rsA[:], in0=rn[0:96, 0:1],
                                        in1=tempcol[0:96, 0:1], op=OP.mult)
                # partition-offset rearrangements go through SBUF->SBUF DMA:
                # DVE writes at non-quadrant-aligned partition bases are illegal
                rsB = ep.tile([96, 1], F32)
                nc.sync.dma_start(out=rsB[0:32, :], in_=rn[96:128, 0:1])
                nc.sync.dma_start(out=rsB[32:96, :], in_=rn[0:64, 1:2])
                nc.vector.tensor_tensor(out=rsB[:], in0=rsB[:],
                                        in1=tempcol[0:96, 1:2], op=OP.mult)

                # column scales (k-norms) -> broadcast [96, 96] via rank-1 matmul
                rkc = ep.tile([96, 2], F32)
                nc.sync.dma_start(out=rkc[0:64, 0:1], in_=rn[64:128, 1:2])
                nc.sync.dma_start(out=rkc[64:96, 0:1], in_=rn[0:32, 2:3])
                nc.sync.dma_start(out=rkc[0:96, 1:2], in_=rn[32:128, 2:3])
                tps = eps.tile([2, 96], F32, tag="t")
                nc.tensor.transpose(tps[:], rkc[:], ident32[0:96, 0:96])
                rkrs = ep.tile([2, 96], F32)
                nc.vector.tensor_copy(rkrs[:], tps[:])
                rkr = [ep.tile([1, 96], F32, name=f"rkr{g}") for g in range(2)]
                nc.vector.tensor_copy(rkr[0][:], rkrs[0:1, :])
                nc.sync.dma_start(out=rkr[1][:], in_=rkrs[1:2, :])
                bcps = eps.tile([96, 96], F32, tag="bc")
                bc = [ep.tile([96, 96], F32, name=f"bc{g}") for g in range(2)]
                for g in range(2):
                    nc.tensor.matmul(bcps[:], ones32[0:1, 0:96], rkr[g][:],
                                     start=True, stop=True)
                    nc.vector.tensor_copy(bc[g][:], bcps[:])
                pe_warm(110, "mid")

                # logits = gram * rq * rk * temp; diag-extract -> [96, 24] per tile
                attn = []
                for g in range(2):
                    lg = ep.tile([96, 96], F32, name=f"lg{g}")
                    nc.vector.tensor_scalar(out=lg[:], in0=statf[0:96, 96 * g:96 * (g + 1)],
                                            scalar1=(rsA if g == 0 else rsB)[:],
                                            scalar2=None, op0=OP.mult)
                    nc.vector.tensor_tensor(out=lg[:], in0=lg[:], in1=bc[g][:], op=OP.mult)
                    sm = ep.tile([96, HD], F32, name=f"sm{g}")
                    for hl in range(4):
                        nc.sync.dma_start(out=sm[24 * hl:24 * (hl + 1), :],
                                          in_=lg[24 * hl:24 * (hl + 1), 24 * hl:24 * (hl + 1)])
                    mx = ep.tile([96, 1], F32, name=f"mx{g}")
                    nc.vector.tensor_reduce(out=mx[:], in_=sm[:], axis=AX.X, op=OP.max)
                    nc.vector.tensor_scalar(out=sm[:], in0=sm[:], scalar1=mx[:],
                                            scalar2=None, op0=OP.subtract)
                    ex = ep.tile([96, HD], F32, name=f"ex{g}")
                    nc.scalar.activation(out=ex[:], in_=sm[:], func=ACTF.Exp)
                    sme = ep.tile([96, 1], F32, name=f"sme{g}")
                    nc.vector.tensor_reduce(out=sme[:], in_=ex[:], axis=AX.X, op=OP.add)
                    rs = ep.tile([96, 1], F32, name=f"rs{g}")
                    nc.vector.reciprocal(rs[:], sme[:])
                    at = ep.tile([96, HD], BF16, name=f"at{g}")
                    nc.vector.tensor_scalar(out=at[:], in0=ex[:], scalar1=rs[:],
                                            scalar2=None, op0=OP.mult)
                    attn.append(at)

                # blockdiag(attn) as lhsT rows=out-chan(24h+d), cols=v-chan(24h+e)
                abd0 = ep.tile([128, C], BF16)
                abd1 = ep.tile([64, C], BF16)
                nc.vector.memset(abd0[:], 0.0)
                nc.vector.memset(abd1[:], 0.0)
                for h in range(HEADS):
                    g, hl = divmod(h, 4)
                    src = attn[g]
                    r0, cc0 = 24 * h, 24 * h
                    if r0 + 24 <= 128:
                        nc.sync.dma_start(out=abd0[r0:r0 + 24, cc0:cc0 + 24],
                                          in_=src[24 * hl:24 * hl + 24, :])
                    elif r0 >= 128:
                        nc.sync.dma_start(out=abd1[r0 - 128:r0 - 104, cc0:cc0 + 24],
                                          in_=src[24 * hl:24 * hl + 24, :])
                    else:
                        k0 = 128 - r0
                        nc.sync.dma_start(out=abd0[r0:128, cc0:cc0 + 24],
                                          in_=src[24 * hl:24 * hl + k0, :])
                        nc.sync.dma_start(out=abd1[0:24 - k0, cc0:cc0 + 24],
                                          in_=src[24 * hl + k0:24 * hl + 24, :])

                # W2T[c, o] = sum_r abd[r, c] * projt[r, o]
                w2t0 = ep.tile([128, C], BF16)
                w2t1d = ep.tile([128, C], BF16)  # [64 vch] duplicated on both halves
                wps = eps.tile([128, C], F32, tag="wps")
                nc.tensor.matmul(wps[:], abd0[:, 0:128], projt0[:], start=True, stop=False)
                nc.tensor.matmul(wps[:], abd1[:, 0:128], projt1[:], start=False, stop=True)
                nc.scalar.copy(w2t0[:], wps[:])
                wps2 = eps.tile([64, C], F32, tag="wps2")
                nc.tensor.matmul(wps2[:], abd0[:, 128:192], projt0[:], start=True, stop=False)
                nc.tensor.matmul(wps2[:], abd1[:, 128:192], projt1[:], start=False, stop=True)
                w2t1s = ep.tile([64, C], BF16)
                nc.scalar.copy(w2t1s[:], wps2[:])
                nc.sync.dma_start(out=w2t1d[0:64, :], in_=w2t1s[:])
                nc.sync.dma_start(out=w2t1d[64:128, :], in_=w2t1s[:])
                pe_warm(60, "ep")

              # ================= phase 2: out = W2 @ v =================
              with tc.tile_pool(name="p2", bufs=2) as p2, \
                   tc.tile_pool(name="ps2", bufs=2, space="PSUM") as ps2, \
                   tc.tile_pool(name="wps2p", bufs=1, space="PSUM") as wps2p:
                  warm2 = wps2p.tile([128, 128], F32)
                  PX2 = 2048
                  for c in range(HALF * W // PX2):
                      cs = slice(c * PX2, (c + 1) * PX2)
                      vb1 = p2.tile([128, PX2], BF16, tag="vb1", bufs=4)
                      hp = PX2 // 2
                      for hh in range(2):
                          hs = slice(c * PX2 + hh * hp, c * PX2 + (hh + 1) * hp)
                          nc.sync.dma_start(out=vb1[0:64, hh * hp:(hh + 1) * hp],
                                            in_=v4_dram[:, hs])
                          nc.sync.dma_start(out=vb1[64:128, hh * hp:(hh + 1) * hp],
                                            in_=v4_dram[:, hs])
                      ob0 = p2.tile([128, PX2], BF16, tag="ob0")
                      ob1 = p2.tile([64, PX2], BF16, tag="ob1")
                      for nb in range(PX2 // 512):
                          nbs = slice(nb * 512, (nb + 1) * 512)
                          gs = slice(c * PX2 + nb * 512, c * PX2 + (nb + 1) * 512)
                          f0 = ps2.tile([128, 512], F32, tag="f0", bufs=3)
                          f1 = ps2.tile([64, 512], F32, tag="f1", bufs=3)
                          nc.tensor.matmul(f0[:], w2t0[:, 0:128], v3res[:, gs],
                                           start=True, stop=False)
                          nc.tensor.matmul(f1[:], w2t0[:, 128:192], v3res[:, gs],
                                           start=True, stop=False)
                          nc.tensor.matmul(f0[:], w2t1d[0:64, 0:128], vb1[0:64, nbs],
                                           start=False, stop=True, tile_position=(0, 0),
                                           skip_group_check=True)
                          nc.tensor.matmul(f1[:], w2t1d[64:128, 128:192], vb1[64:128, nbs],
                                           start=False, stop=True, tile_position=(64, 0),
                                           skip_group_check=True)
                          nc.scalar.copy(ob0[:, nbs], f0[:])
                          nc.vector.tensor_copy(ob1[:, nbs], f1[:])
                      for _ in range(6):  # keep the PE activity monitor busy
                          nc.tensor.matmul(warm2[:], ident[:], ident[:],
                                           start=True, stop=True)
                      nc.sync.dma_start(out=out_ext[0:128, cs], in_=ob0[:])
                      nc.sync.dma_start(out=out_ext[128:192, cs], in_=ob1[:])
    return nc


_NC_CACHE = None


def _get_nc():
    global _NC_CACHE
    if _NC_CACHE is None:
        _NC_CACHE = build_nc()
    return _NC_CACHE


def _shard_inputs(x, qkv_w, dw_w, proj_w, temperature):
    w9 = np.asarray(dw_w, np.float64).reshape(C3, 9)
    w11 = w9[:, 4].copy()
    # clamp |w11| away from 0 so the 1/w11 recovery stays finite
    w11 = np.where(np.abs(w11) < 1e-20, 1e-20, w11)
    qkvwt = np.ascontiguousarray((np.asarray(qkv_w, np.float64) * w11[:, None]).T)
    qkvwt = qkvwt.astype(np.float32).astype(ml_dtypes.bfloat16)
    projt = np.ascontiguousarray(np.asarray(proj_w).T).astype(ml_dtypes.bfloat16)

    # per-PE-tap diagonal weight blocks: dwdiag[s, i, CTO[ct]+i] = w(tap_s, ch)
    dwdiag = np.zeros((len(PE_TAPS), 128, C3), np.float32)
    for s, (dy, dx) in enumerate(PE_TAPS):
        wv = w9[:, 3 * dy + dx]
        for ct in range(5):
            idx = np.arange(CT[ct])
            dwdiag[s, idx, CTO[ct] + idx] = wv[CTO[ct] + idx]
    dwdiag = dwdiag.reshape(len(PE_TAPS) * 128, C3).astype(ml_dtypes.bfloat16)

    # ct4 pair weights: [diag(w_tapA); diag(w_tapB)] as [128, 64] lhsT blocks
    # pair 0 = ((0,0),(0,2)) -> w9 cols 0, 2; pair 1 = ((1,0),(1,2)) -> 3, 5
    dwpair = np.zeros((2, 128, 64), np.float32)
    for g, (ka, kb) in enumerate(((0, 2), (3, 5))):
        idx = np.arange(64)
        dwpair[g, idx, idx] = w9[512 + idx, ka]
        dwpair[g, 64 + idx, idx] = w9[512 + idx, kb]
    dwpair = dwpair.reshape(256, 64).astype(ml_dtypes.bfloat16)

    # DVE tap columns per ct: w(0,1), w(2,1), w(2,2)
    dvecol = np.zeros((128, 15), np.float32)
    recip11 = np.zeros((128, 5), np.float32)
    for ct in range(5):
        idx = np.arange(CT[ct])
        dvecol[idx, 3 * ct + 0] = w9[CTO[ct] + idx, 1]
        dvecol[idx, 3 * ct + 1] = w9[CTO[ct] + idx, 7]
        dvecol[idx, 3 * ct + 2] = w9[CTO[ct] + idx, 8]
        recip11[idx, ct] = (1.0 / w11[CTO[ct] + idx]).astype(np.float32)

    temp = np.asarray(temperature).reshape(HEADS)
    tempcol = np.zeros((128, 2), np.float32)
    for h in range(HEADS):
        g, hl = divmod(h, 4)
        tempcol[24 * hl:24 * (hl + 1), g] = temp[h]

    in_maps = []
    for i in range(8):
        b, hf = divmod(i, 2)
        xin = np.zeros((C, HALF + 2, W), np.float32)
        r0 = hf * HALF - 1
        lo, hi = max(r0, 0), min(r0 + HALF + 2, H)
        xin[:, lo - r0:hi - r0, :] = x[b, :, lo:hi, :]
        in_maps.append({
            "xin": xin.reshape(C, (HALF + 2) * W).astype(ml_dtypes.bfloat16),
            "qkvwt": qkvwt, "projt": projt, "dwdiag": dwdiag,
            "dwpair": dwpair, "dvecol": dvecol, "recip11": recip11,
            "tempcol": tempcol,
        })
    return in_maps


def kernel(x, qkv_w, dw_w, proj_w, temperature):
    nc = _get_nc()
    in_maps = _shard_inputs(x, qkv_w, dw_w, proj_w, temperature)
    res = run_bass_kernel_spmd(nc, in_maps, core_ids=list(range(8)))
    out = np.empty((B, C, H, W), np.float32)
    for i in range(8):
        b, hf = divmod(i, 2)
        o = res.results[i]["out"].astype(np.float32).reshape(C, HALF, W)
        out[b, :, hf * HALF:(hf + 1) * HALF, :] = o
    return out


# revision 28
# speedup vs baseline: 1.2052x; 1.2052x over previous
"""XCA-style attention block (qkv 1x1 conv -> depthwise 3x3 -> L2-normed
cross-covariance attention -> 1x1 proj) on 8 TRN2 NeuronCores.

Sharding: core i handles (batch b = i//2, image half hf = i%2): 128 rows of
the 256-row image. Per-pair [128,195] stats all-reduce; softmax + temperature
+ norm fixups fold into W2 = proj @ blockdiag(attn); phase 2 = W2 @ v.

Phase-1 redesign vs the old kernel: 32 chunks x 4 output rows, each
self-contained (qkv recomputed for the 2 halo rows). The 3x3 center tap
(1,1) is folded into the qkv weights (psum = w11*qkv; raw qkv recovered by
the ACT evacuation with scale=1/w11 into a row-padded "ring" buffer: 260
stride, zero gap cols => no wrap patches, no shifted qB copy). Five side
taps run as per-row N=256 diagonal matmuls accumulating onto the same psum.
Tap (2,2) folds into the DVE scalar_tensor_tensor psum evacuation; taps
(0,1),(2,1) are aligned DVE tensor_scalar+tensor_tensor pairs. v channels
384..511 stay resident in SBUF for phase 2; 512..575 spill to DRAM.
"""
import numpy as np
import ml_dtypes

import concourse.bass as bass
import concourse.tile as tile
from concourse import mybir
from concourse.bass_utils import run_bass_kernel_spmd
from concourse.masks import make_identity

# --- patch: this walrus build rejects >1 semaphore wait on a Drain ---------
import concourse.tile as _tile_mod
from concourse.vector_clock import ScopedClock as _SC, VectorClock as _VC


def _drain_and_barrier(self, tick_clock, wait_clock):
    gc = tick_clock.global_clock
    n = len(gc)
    nonzero = [i for i in range(n) if gc[i] > 0]
    for i in nonzero:
        vec = [gc[j] if j == i else 0 for j in range(n)]
        inst = self.nc.sync.drain()
        wait_clock.add_sem_waits(inst.ins, _SC({None: _VC(vec)}))
    if not nonzero:
        inst = self.nc.sync.drain()
        wait_clock.add_sem_waits(inst.ins, _SC({None: gc}))
    self.nc.all_engine_barrier()
    assert self.sems is not None
    popped = self.nc._tile_sem_poison_stack.pop()
    assert popped is self._sem_poison
    self.nc.clear_and_free_semaphores(list(self.sems.allocated().values()))
    self.nc.all_engine_barrier()


_tile_mod.TileContext._drain_and_barrier = _drain_and_barrier

# The same walrus limit applies to every engine instruction: at most ONE
# semaphore wait. Split extra waits onto preceding same-engine NoOps (engines
# execute in order, so earlier waits still gate the instruction). DMA copies
# use the descriptor path and tolerate multiple waits, so leave them alone.
_orig_commit_and_lower = _tile_mod.TileContext._commit_and_lower
_split_counter = [0]


def _commit_and_lower_split(self, inst, original_block, old_bb_map, bb_to_exit_bb):
    si = getattr(inst, "sync_info", None)
    if si is not None and len(si.on_wait) > 1 and inst.engine is not None:
        waits = list(si.on_wait)
        for w in waits[:-1]:
            _split_counter[0] += 1
            nop = mybir.InstNoOp(
                name=f"{inst.name}-wsplit{_split_counter[0]}",
                sync_info=mybir.SyncInfo(on_wait=[w], on_update=[]),
                bass_nofuse=True,
                engine=inst.engine,
            )
            self._commit_instruction(nop)
        inst.sync_info = mybir.SyncInfo(on_wait=[waits[-1]], on_update=list(si.on_update))
    return _orig_commit_and_lower(self, inst, original_block, old_bb_map, bb_to_exit_bb)


_tile_mod.TileContext._commit_and_lower = _commit_and_lower_split
# ---------------------------------------------------------------------------

F32 = mybir.dt.float32
BF16 = mybir.dt.bfloat16
AX = mybir.AxisListType
OP = mybir.AluOpType
ACTF = mybir.ActivationFunctionType

B, C, H, W = 4, 192, 256, 256
HEADS, HD = 8, 24
C3 = 3 * C  # 576
HALF = H // 2  # rows per core
CH = 4  # output rows per chunk
NCH = HALF // CH  # 32
RIN = CH + 2  # rows incl halo
SW = W + 4  # padded ring row stride (260)
PXC = CH * W  # 1024 px per chunk
CT = [128, 128, 128, 128, 64]
CTO = [0, 128, 256, 384, 512]
NIT = NCH * 5  # flattened (chunk, ct) iterations
PAIRS = [[0, 1], [2, 3], [4, 5], [6, 7]]
CCSPLIT = 26  # chunks [0, CCSPLIT) all-reduce early, hidden behind the tail

# PE side taps (dy, dx); (1,1) folded into qkv weights, (2,2) folded into
# the DVE stt evacuation, (0,1)/(2,1) are DVE ts+tt pairs.
PE_TAPS = [(0, 0), (0, 2), (1, 0), (1, 2), (2, 0)]


def build_nc():
    nc = bass.Bass()
    x_ext = nc.declare_dram_parameter("xin", [C, (HALF + 2) * W], BF16, isOutput=False)
    qkvwt_ext = nc.declare_dram_parameter("qkvwt", [C, C3], BF16, isOutput=False)
    projt_ext = nc.declare_dram_parameter("projt", [C, C], BF16, isOutput=False)
    dwd_ext = nc.declare_dram_parameter("dwdiag", [len(PE_TAPS) * 128, C3], BF16, isOutput=False)
    dwpair_ext = nc.declare_dram_parameter("dwpair", [2 * 128, 64], BF16, isOutput=False)
    dvecol_ext = nc.declare_dram_parameter("dvecol", [128, 15], F32, isOutput=False)
    recip_ext = nc.declare_dram_parameter("recip11", [128, 5], F32, isOutput=False)
    tempcol_ext = nc.declare_dram_parameter("tempcol", [128, 2], F32, isOutput=False)
    out_ext = nc.declare_dram_parameter("out", [C, HALF * W], BF16, isOutput=True)

    with tile.TileContext(nc) as tc:
        with tc.tile_pool(name="wpool", bufs=1) as wp, \
             tc.tile_pool(name="dram", bufs=1, space="DRAM") as dram:
            # ---- weights / constants
            qkvw0 = wp.tile([128, C3], BF16)
            qkvw1 = wp.tile([128, C3], BF16)
            nc.sync.dma_start(out=qkvw0[:], in_=qkvwt_ext[0:128, :])
            nc.sync.dma_start(out=qkvw1[0:64, :], in_=qkvwt_ext[128:192, :])
            nc.sync.dma_start(out=qkvw1[64:128, :], in_=qkvwt_ext[128:192, :])
            projt0 = wp.tile([128, C], BF16)
            projt1 = wp.tile([64, C], BF16)
            nc.sync.dma_start(out=projt0[:], in_=projt_ext[0:128, :])
            nc.sync.dma_start(out=projt1[:], in_=projt_ext[128:192, :])
            dwd = [wp.tile([128, C3], BF16, name=f"dwd{s}") for s in range(len(PE_TAPS))]
            for s in range(len(PE_TAPS)):
                nc.sync.dma_start(out=dwd[s][:], in_=dwd_ext[128 * s:128 * (s + 1), :])
            dwpair = [wp.tile([128, 64], BF16, name=f"dwp{g}") for g in range(2)]
            for g in range(2):
                nc.sync.dma_start(out=dwpair[g][:], in_=dwpair_ext[128 * g:128 * (g + 1), :])
            dvecol = wp.tile([128, 15], F32)
            nc.sync.dma_start(out=dvecol[:], in_=dvecol_ext[:])
            recip11 = wp.tile([128, 5], F32)
            nc.sync.dma_start(out=recip11[:], in_=recip_ext[:])
            tempcol = wp.tile([128, 2], F32)
            nc.sync.dma_start(out=tempcol[:], in_=tempcol_ext[:])
            ident = wp.tile([128, 128], BF16)
            make_identity(nc, ident[:])
            ident32 = wp.tile([128, 128], F32)
            make_identity(nc, ident32[:])
            ones32 = wp.tile([1, 128], F32)
            nc.vector.memset(ones32[:], 1.0)

            # persistent accumulators / residents
            sq_part = [wp.tile([CT[ct], NCH], F32, name=f"sqp{ct}") for ct in range(3)]
            v3res = wp.tile([128, HALF * W], BF16)  # v channels 384..511
            v4_dram = dram.tile([64, HALF * W], BF16)
            stats = wp.tile([128, 195], F32)
            stats2 = wp.tile([128, 195], F32)
            statf1 = wp.tile([128, 195], F32)
            cc_in = dram.tile([128, 195], F32)
            cc_out = dram.tile([128, 195], F32)
            cc_in2 = dram.tile([128, 195], F32)
            cc_out2 = dram.tile([128, 195], F32)

            def pe_warm(n, tag):
                # dummy matmuls keeping the PE HAM busy across PE-idle spans
                # (DMA prologue/collective/epilogue) so real matmuls run at 2.4 GHz
                with tc.tile_pool(name=f"wm{tag}", bufs=1, space="PSUM") as wpp:
                    wps = wpp.tile([128, 128], F32)
                    for _ in range(n):
                        nc.tensor.matmul(wps[:], ident[:], ident[:],
                                         start=True, stop=True)

            pe_warm(350, "start")

            # ================= phase 1 =================
            with tc.tile_pool(name="p1", bufs=2) as p1, \
                 tc.tile_pool(name="ps1", bufs=2, space="PSUM") as ps1, \
                 tc.tile_pool(name="gps", bufs=1, space="PSUM") as gps:
                gAB = gps.tile([96, 192], F32, tag="gAB")
                gA = gAB[:, 0:96]
                gB = gAB[:, 96:192]

                state = {}  # per live iteration: ring view, psum, etc.

                def issue_qkv(it):
                    c, ct = divmod(it, 5)
                    cw = CT[ct]
                    cts = slice(CTO[ct], CTO[ct] + cw)
                    r0 = 4 * c  # first xin row of the 6-row window
                    # x tiles for this chunk are shared across cts via tags
                    if ct == 0:
                        x0 = p1.tile([128, RIN * W], BF16, tag="x0", bufs=3)
                        x1 = p1.tile([128, RIN * W], BF16, tag="x1", bufs=3)
                        nc.sync.dma_start(out=x0[:], in_=x_ext[0:128, r0 * W:(r0 + RIN) * W])
                        nc.sync.dma_start(out=x1[0:64, :], in_=x_ext[128:192, r0 * W:(r0 + RIN) * W])
                        nc.sync.dma_start(out=x1[64:128, :], in_=x_ext[128:192, r0 * W:(r0 + RIN) * W])
                        state[("x", c)] = (x0, x1)
                    x0, x1 = state[("x", c)]

                    Bm = ps1.tile([cw, 4 * W], F32, tag="Bm", name=f"Bm{ct}")
                    Bh = ps1.tile([cw, 2 * W], F32, tag="Bh", bufs=1, name=f"Bh{ct}")
                    # main rows (xin rows r0+1..r0+4): two 512 blocks
                    nc.tensor.matmul(Bm[:, 0:512], qkvw0[:, cts], x0[:, W:3 * W],
                                     start=True, stop=False)
                    nc.tensor.matmul(Bm[:, 512:1024], qkvw0[:, cts], x0[:, 3 * W:5 * W],
                                     start=True, stop=False)
                    nc.tensor.matmul(Bm[:, 0:512], qkvw1[0:64, cts], x1[0:64, W:3 * W],
                                     start=False, stop=False, tile_position=(0, 0),
                                     skip_group_check=True)
                    nc.tensor.matmul(Bm[:, 512:1024], qkvw1[64:128, cts], x1[64:128, 3 * W:5 * W],
                                     start=False, stop=False, tile_position=(64, 0),
                                     skip_group_check=True)
                    x0v = x0[:].rearrange("p (r w) -> p r w", w=W)
                    x1v = x1[:].rearrange("p (r w) -> p r w", w=W)
                    if c == 0:
                        # halo rows r0, r0+5 -> Bh (strided 2-row rhs view)
                        nc.tensor.matmul(Bh[:], qkvw0[:, cts], x0v[:, 0:6:5, :],
                                         start=True, stop=False)
                        nc.tensor.matmul(Bh[:], qkvw1[0:64, cts], x1v[0:64, 0:6:5, :],
                                         start=False, stop=True, tile_position=(0, 0),
                                         skip_group_check=True)
                    else:
                        # only row r0+5; row r0 is copied from the previous ring
                        nc.tensor.matmul(Bh[:, 256:512], qkvw0[:, cts], x0v[:, 5, :],
                                         start=True, stop=False)
                        nc.tensor.matmul(Bh[:, 256:512], qkvw1[0:64, cts], x1v[0:64, 5, :],
                                         start=False, stop=True, tile_position=(0, 0),
                                         skip_group_check=True)

                    # ring: padded raw-qkv rows (scale 1/w11). ct4 has a +2-col
                    # shifted duplicate on partitions 64:128 for tap pairing.
                    prows = 128 if ct == 4 else cw
                    ring = p1.tile([prows, RIN * SW], BF16, tag=f"ring{ct}", name=f"ring{ct}")
                    if c < 2:
                        nc.gpsimd.memset(ring[:], 0.0)
                    rv = ring[0:cw].rearrange("p (r w) -> p r w", w=SW)
                    rsc = recip11[0:cw, ct:ct + 1]
                    Bmv = Bm[:].rearrange("p (r w) -> p r w", w=W)
                    nc.scalar.activation(out=rv[:, 1:5, 2:2 + W], in_=Bmv,
                                         func=ACTF.Copy, scale=rsc)
                    if c == 0:
                        Bhv = Bh[:].rearrange("p (r w) -> p r w", w=W)
                        nc.scalar.activation(out=rv[:, 0:6:5, 2:2 + W], in_=Bhv,
                                             func=ACTF.Copy, scale=rsc)
                    else:
                        nc.scalar.activation(out=rv[:, 5:6, 2:2 + W],
                                             in_=Bh[:, 256:512],
                                             func=ACTF.Copy, scale=rsc)
                        rvp = state[("rv", c - 1, ct)]
                        nc.sync.dma_start(out=rv[:, 0, 2:2 + W], in_=rvp[:, 4, 2:2 + W])
                    state[("rv", c, ct)] = rv
                    state.pop(("rv", c - 2, ct), None)
                    if ct == 4:
                        # dup partitions 64:128 = data shifted +2 cols
                        r4f = ring[:].rearrange("p (r w) -> p r w", w=SW)
                        nc.sync.dma_start(out=r4f[64:128, :, 0:SW - 2],
                                          in_=r4f[0:64, :, 2:SW])
                        state[("r4", it)] = r4f
                    state[("B", it)] = (Bm, ring, rv)

                def issue_consume(it):
                    c, ct = divmod(it, 5)
                    cw = CT[ct]
                    cts = slice(CTO[ct], CTO[ct] + cw)
                    Bm, ring, rv = state.pop(("B", it))

                    # 5 side taps: N=512 2-row diag MMs (strided rhs), tap-outer.
                    # ct4 packs ((0,0),(0,2)) and ((1,0),(1,2)) via the +2-col
                    # dup on partitions 64:128 (one 128-contraction MM each).
                    if ct == 4:
                        r4f = state.pop(("r4", it))
                        for g, dy in ((0, 0), (1, 1)):
                            for b in range(2):
                                nc.tensor.matmul(
                                    Bm[:, b * 512:(b + 1) * 512], dwpair[g][:],
                                    r4f[:, 2 * b + dy:2 * b + dy + 2, 1:1 + W],
                                    start=False, stop=False,
                                    skip_group_check=True)
                        for b in range(2):
                            nc.tensor.matmul(
                                Bm[:, b * 512:(b + 1) * 512], dwd[4][0:cw, cts],
                                rv[:, 2 * b + 2:2 * b + 4, 1:1 + W],
                                start=False, stop=True,
                                skip_group_check=True)
                    else:
                        for s, (dy, dx) in enumerate(PE_TAPS):
                            lw = dwd[s][0:cw, cts]
                            for b in range(2):
                                nc.tensor.matmul(
                                    Bm[:, b * 512:(b + 1) * 512], lw,
                                    rv[:, 2 * b + dy:2 * b + dy + 2, 1 + dx:1 + dx + W],
                                    start=False, stop=(s == len(PE_TAPS) - 1),
                                    skip_group_check=True)

                    # evacuate + fold tap (2,2): acc = ring(2,2)*w22 + psum
                    if ct == 3:
                        acc = v3res[:, c * PXC:(c + 1) * PXC]
                    else:
                        acc_t = p1.tile([cw, PXC], BF16, tag=f"acc{ct}", name=f"acc{ct}")
                        acc = acc_t[:]
                    nc.vector.scalar_tensor_tensor(
                        out=acc, in0=rv[:, 2:6, 3:3 + W],
                        scalar=dvecol[0:cw, 3 * ct + 2:3 * ct + 3],
                        in1=Bm[:], op0=OP.mult, op1=OP.add)
                    # taps (0,1) and (2,1): ts mult (DVE) + tt add (DVE / GpSimd)
                    for k, dy in ((0, 0), (1, 2)):
                        tmp = p1.tile([cw, PXC], BF16, tag="dwtmp", name="dwtmp")
                        nc.vector.tensor_scalar(
                            out=tmp[:], in0=rv[:, dy:dy + 4, 2:2 + W],
                            scalar1=dvecol[0:cw, 3 * ct + k:3 * ct + k + 1],
                            scalar2=None, op0=OP.mult)
                        nc.vector.tensor_tensor(out=acc, in0=acc, in1=tmp[:], op=OP.add)

                    if ct < 3:
                        scr = p1.tile([cw, PXC], BF16, tag="sqscr")
                        nc.scalar.activation(out=scr[:], in_=acc, func=ACTF.Square,
                                             accum_out=sq_part[ct][:, c:c + 1])
                        state[("acc", c, ct)] = acc
                    if ct == 4:
                        nc.sync.dma_start(out=v4_dram[:, c * PXC:(c + 1) * PXC], in_=acc)

                def issue_gram(c):
                    accs = [state.pop(("acc", c, ct)) for ct in range(3)]
                    # 8 px-blocks of 128; 2 pb per tg/qkt tile
                    for g in range(4):
                        tg = gps.tile([128, 768], BF16, tag="tg", bufs=2)
                        qkt = p1.tile([128, 768], BF16, tag="qkt", bufs=3)
                        for h in range(2):
                            pb = 2 * g + h
                            pbs = slice(pb * 128, (pb + 1) * 128)
                            for q in range(3):
                                nc.tensor.transpose(
                                    tg[:, h * 384 + q * 128:h * 384 + (q + 1) * 128],
                                    accs[q][:, pbs], ident[:])
                        nc.scalar.copy(qkt[:], tg[:])
                        for h in range(2):
                            o = h * 384
                            first = (c in (0, CCSPLIT) and g == 0 and h == 0)
                            last = (c in (CCSPLIT - 1, NCH - 1) and g == 3 and h == 1)
                            nc.tensor.matmul(gA, qkt[:, o:o + 96], qkt[:, o + 192:o + 288],
                                             start=first, stop=last)
                            nc.tensor.matmul(gB, qkt[:, o + 96:o + 192], qkt[:, o + 288:o + 384],
                                             start=first, stop=last)

                for it in range(NIT + 1):
                    if it < NIT:
                        issue_qkv(it)
                    if it >= 1:
                        issue_consume(it - 1)
                        c, ct = divmod(it - 1, 5)
                        if ct == 3:
                            issue_gram(c)
                        if ct == 3 and c == CCSPLIT - 1:
                            # part-A stats all-reduce, hidden behind the
                            # remaining chunks
                            for q in range(3):
                                nc.vector.tensor_reduce(
                                    out=stats[0:CT[q], 192 + q:193 + q],
                                    in_=sq_part[q][:, 0:CCSPLIT], axis=AX.X, op=OP.add)
                            nc.scalar.copy(stats[0:96, 0:192], gAB[:])
                            nc.sync.dma_start(out=cc_in[:], in_=stats[:])
                            nc.gpsimd.collective_compute(
                                "AllReduce", OP.add, replica_groups=PAIRS,
                                ins=[cc_in.opt()], outs=[cc_out.opt()])
                            nc.sync.dma_start(out=statf1[:], in_=cc_out[:])

                # part-B stats; pack stats2 = [gA | gB | ssq]
                for ct in range(3):
                    nc.vector.tensor_reduce(
                        out=stats2[0:CT[ct], 192 + ct:193 + ct],
                        in_=sq_part[ct][:, CCSPLIT:NCH], axis=AX.X, op=OP.add)
                nc.scalar.copy(stats2[0:96, 0:192], gAB[:])

            # ================= collective (part B) =================
            nc.sync.dma_start(out=cc_in2[:], in_=stats2[:])
            nc.gpsimd.collective_compute(
                "AllReduce", OP.add, replica_groups=PAIRS,
                ins=[cc_in2.opt()], outs=[cc_out2.opt()])
            statf = wp.tile([128, 195], F32)
            nc.sync.dma_start(out=statf[:], in_=cc_out2[:])
            nc.vector.tensor_tensor(out=statf[:], in0=statf[:], in1=statf1[:], op=OP.add)
            pe_warm(400, "cc")

            # ================= epilogue (tiny) =================
            with tc.tile_pool(name="ep", bufs=1) as ep:
              with tc.tile_pool(name="eps", bufs=1, space="PSUM") as eps:
                # 1/max(sqrt(ssq), eps) per q/k channel, [128, 3] by ct
                nrm = ep.tile([128, 3], F32)
                nc.scalar.activation(out=nrm[:], in_=statf[:, 192:195], func=ACTF.Sqrt)
                nc.vector.tensor_scalar(out=nrm[:], in0=nrm[:], scalar1=1e-12,
                                        scalar2=None, op0=OP.max)
                rn = ep.tile([128, 3], F32)
                nc.vector.reciprocal(rn[:], nrm[:])

                # row scales (q-norms * temperature), partition-packed per gram tile
                rsA = ep.tile([96, 1], F32)
                nc.vector.tensor_tensor(out=rsA[:], in0=rn[0:96, 0:1],
                                        in1=tempcol[0:96, 0:1], op=OP.mult)
                # partition-offset rearrangements go through SBUF->SBUF DMA:
                # DVE writes at non-quadrant-aligned partition bases are illegal
                rsB = ep.tile([96, 1], F32)
                nc.sync.dma_start(out=rsB[0:32, :], in_=rn[96:128, 0:1])
                nc.sync.dma_start(out=rsB[32:96, :], in_=rn[0:64, 1:2])
                nc.vector.tensor_tensor(out=rsB[:], in0=rsB[:],
                                        in1=tempcol[0:96, 1:2], op=OP.mult)

                # column scales (k-norms) -> broadcast [96, 96] via rank-1 matmul
                rkc = ep.tile([96, 2], F32)
                nc.sync.dma_start(out=rkc[0:64, 0:1], in_=rn[64:128, 1:2])
                nc.sync.dma_start(out=rkc[64:96, 0:1], in_=rn[0:32, 2:3])
                nc.sync.dma_start(out=rkc[0:96, 1:2], in_=rn[32:128, 2:3])
                tps = eps.tile([2, 96], F32, tag="t")
                nc.tensor.transpose(tps[:], rkc[:], ident32[0:96, 0:96])
                rkrs = ep.tile([2, 96], F32)
                nc.vector.tensor_copy(rkrs[:], tps[:])
                rkr = [ep.tile([1, 96], F32, name=f"rkr{g}") for g in range(2)]
                nc.vector.tensor_copy(rkr[0][:], rkrs[0:1, :])
                nc.sync.dma_start(out=rkr[1][:], in_=rkrs[1:2, :])
                bcps = eps.tile([96, 96], F32, tag="bc")
                bc = [ep.tile([96, 96], F32, name=f"bc{g}") for g in range(2)]
                for g in range(2):
                    nc.tensor.matmul(bcps[:], ones32[0:1, 0:96], rkr[g][:],
                                     start=True, stop=True)
                    nc.vector.tensor_copy(bc[g][:], bcps[:])
                pe_warm(110, "mid")

                # logits = gram * rq * rk * temp; diag-extract -> [96, 24] per tile
                attn = []
                for g in range(2):
                    lg = ep.tile([96, 96], F32, name=f"lg{g}")
                    nc.vector.tensor_scalar(out=lg[:], in0=statf[0:96, 96 * g:96 * (g + 1)],
                                            scalar1=(rsA if g == 0 else rsB)[:],
                                            scalar2=None, op0=OP.mult)
                    nc.vector.tensor_tensor(out=lg[:], in0=lg[:], in1=bc[g][:], op=OP.mult)
                    sm = ep.tile([96, HD], F32, name=f"sm{g}")
                    for hl in range(4):
                        nc.sync.dma_start(out=sm[24 * hl:24 * (hl + 1), :],
                                          in_=lg[24 * hl:24 * (hl + 1), 24 * hl:24 * (hl + 1)])
                    mx = ep.tile([96, 1], F32, name=f"mx{g}")
                    nc.vector.tensor_reduce(out=mx[:], in_=sm[:], axis=AX.X, op=OP.max)
                    nc.vector.tensor_scalar(out=sm[:], in0=sm[:], scalar1=mx[:],
                                            scalar2=None, op0=OP.subtract)
                    ex = ep.tile([96, HD], F32, name=f"ex{g}")
                    nc.scalar.activation(out=ex[:], in_=sm[:], func=ACTF.Exp)
                    sme = ep.tile([96, 1], F32, name=f"sme{g}")
                    nc.vector.tensor_reduce(out=sme[:], in_=ex[:], axis=AX.X, op=OP.add)
                    rs = ep.tile([96, 1], F32, name=f"rs{g}")
                    nc.vector.reciprocal(rs[:], sme[:])
                    at = ep.tile([96, HD], BF16, name=f"at{g}")
                    nc.vector.tensor_scalar(out=at[:], in0=ex[:], scalar1=rs[:],
                                            scalar2=None, op0=OP.mult)
                    attn.append(at)

                # blockdiag(attn) as lhsT rows=out-chan(24h+d), cols=v-chan(24h+e)
                abd0 = ep.tile([128, C], BF16)
                abd1 = ep.tile([64, C], BF16)
                nc.vector.memset(abd0[:], 0.0)
                nc.vector.memset(abd1[:], 0.0)
                for h in range(HEADS):
                    g, hl = divmod(h, 4)
                    src = attn[g]
                    r0, cc0 = 24 * h, 24 * h
                    if r0 + 24 <= 128:
                        nc.sync.dma_start(out=abd0[r0:r0 + 24, cc0:cc0 + 24],
                                          in_=src[24 * hl:24 * hl + 24, :])
                    elif r0 >= 128:
                        nc.sync.dma_start(out=abd1[r0 - 128:r0 - 104, cc0:cc0 + 24],
                                          in_=src[24 * hl:24 * hl + 24, :])
                    else:
                        k0 = 128 - r0
                        nc.sync.dma_start(out=abd0[r0:128, cc0:cc0 + 24],
                                          in_=src[24 * hl:24 * hl + k0, :])
                        nc.sync.dma_start(out=abd1[0:24 - k0, cc0:cc0 + 24],
                                          in_=src[24 * hl + k0:24 * hl + 24, :])

                # W2T[c, o] = sum_r abd[r, c] * projt[r, o]; out columns padded
                # to 256 with zeros so phase-2 f1 matmuls are full-array
                w2t0 = ep.tile([128, 256], BF16)
                w2t1d = ep.tile([128, 256], BF16)  # [64 vch] dup'd on both halves
                nc.vector.memset(w2t0[:, 192:256], 0.0)
                nc.vector.memset(w2t1d[:, 192:256], 0.0)
                wps = eps.tile([128, C], F32, tag="wps")
                nc.tensor.matmul(wps[:], abd0[:, 0:128], projt0[:], start=True, stop=False)
                nc.tensor.matmul(wps[:], abd1[:, 0:128], projt1[:], start=False, stop=True)
                nc.scalar.copy(w2t0[:, 0:C], wps[:])
                wps2 = eps.tile([64, C], F32, tag="wps2")
                nc.tensor.matmul(wps2[:], abd0[:, 128:192], projt0[:], start=True, stop=False)
                nc.tensor.matmul(wps2[:], abd1[:, 128:192], projt1[:], start=False, stop=True)
                w2t1s = ep.tile([64, C], BF16)
                nc.scalar.copy(w2t1s[:], wps2[:])
                nc.sync.dma_start(out=w2t1d[0:64, 0:C], in_=w2t1s[:])
                nc.sync.dma_start(out=w2t1d[64:128, 0:C], in_=w2t1s[:])
                pe_warm(60, "ep")

              # ================= phase 2: out = W2 @ v =================
              with tc.tile_pool(name="p2", bufs=2) as p2, \
                   tc.tile_pool(name="ps2", bufs=2, space="PSUM") as ps2, \
                   tc.tile_pool(name="wps2p", bufs=1, space="PSUM") as wps2p:
                  warm2 = wps2p.tile([128, 128], F32)
                  PX2 = 2048
                  for c in range(HALF * W // PX2):
                      cs = slice(c * PX2, (c + 1) * PX2)
                      vb1 = p2.tile([128, PX2], BF16, tag="vb1", bufs=4)
                      hp = PX2 // 2
                      for hh in range(2):
                          hs = slice(c * PX2 + hh * hp, c * PX2 + (hh + 1) * hp)
                          nc.sync.dma_start(out=vb1[0:64, hh * hp:(hh + 1) * hp],
                                            in_=v4_dram[:, hs])
                          nc.sync.dma_start(out=vb1[64:128, hh * hp:(hh + 1) * hp],
                                            in_=v4_dram[:, hs])
                      ob0 = p2.tile([128, PX2], BF16, tag="ob0")
                      ob1 = p2.tile([64, PX2], BF16, tag="ob1")
                      for nb in range(PX2 // 512):
                          nbs = slice(nb * 512, (nb + 1) * 512)
                          gs = slice(c * PX2 + nb * 512, c * PX2 + (nb + 1) * 512)
                          f0 = ps2.tile([128, 512], F32, tag="f0", bufs=3)
                          f1 = ps2.tile([128, 512], F32, tag="f1", bufs=3)
                          nc.tensor.matmul(f0[:], w2t0[:, 0:128], v3res[:, gs],
                                           start=True, stop=False)
                          nc.tensor.matmul(f1[:], w2t0[:, 128:256], v3res[:, gs],
                                           start=True, stop=False)
                          nc.tensor.matmul(f0[:], w2t1d[0:64, 0:128], vb1[0:64, nbs],
                                           start=False, stop=True, tile_position=(0, 0),
                                           skip_group_check=True)
                          nc.tensor.matmul(f1[:], w2t1d[64:128, 128:256], vb1[64:128, nbs],
                                           start=False, stop=True, tile_position=(64, 0),
                                           skip_group_check=True)
                          nc.scalar.copy(ob0[:, nbs], f0[:])
                          nc.vector.tensor_copy(ob1[:, nbs], f1[0:64, :])
                      for _ in range(6):  # keep the PE activity monitor busy
                          nc.tensor.matmul(warm2[:], ident[:], ident[:],
                                           start=True, stop=True)
                      nc.sync.dma_start(out=out_ext[0:128, cs], in_=ob0[:])
                      nc.sync.dma_start(out=out_ext[128:192, cs], in_=ob1[:])
    return nc


_NC_CACHE = None


def _get_nc():
    global _NC_CACHE
    if _NC_CACHE is None:
        _NC_CACHE = build_nc()
    return _NC_CACHE


def _shard_inputs(x, qkv_w, dw_w, proj_w, temperature):
    w9 = np.asarray(dw_w, np.float64).reshape(C3, 9)
    w11 = w9[:, 4].copy()
    # clamp |w11| away from 0 so the 1/w11 recovery stays finite
    w11 = np.where(np.abs(w11) < 1e-20, 1e-20, w11)
    qkvwt = np.ascontiguousarray((np.asarray(qkv_w, np.float64) * w11[:, None]).T)
    qkvwt = qkvwt.astype(np.float32).astype(ml_dtypes.bfloat16)
    projt = np.ascontiguousarray(np.asarray(proj_w).T).astype(ml_dtypes.bfloat16)

    # per-PE-tap diagonal weight blocks: dwdiag[s, i, CTO[ct]+i] = w(tap_s, ch)
    dwdiag = np.zeros((len(PE_TAPS), 128, C3), np.float32)
    for s, (dy, dx) in enumerate(PE_TAPS):
        wv = w9[:, 3 * dy + dx]
        for ct in range(5):
            idx = np.arange(CT[ct])
            dwdiag[s, idx, CTO[ct] + idx] = wv[CTO[ct] + idx]
    dwdiag = dwdiag.reshape(len(PE_TAPS) * 128, C3).astype(ml_dtypes.bfloat16)

    # ct4 pair weights: [diag(w_tapA); diag(w_tapB)] as [128, 64] lhsT blocks
    # pair 0 = ((0,0),(0,2)) -> w9 cols 0, 2; pair 1 = ((1,0),(1,2)) -> 3, 5
    dwpair = np.zeros((2, 128, 64), np.float32)
    for g, (ka, kb) in enumerate(((0, 2), (3, 5))):
        idx = np.arange(64)
        dwpair[g, idx, idx] = w9[512 + idx, ka]
        dwpair[g, 64 + idx, idx] = w9[512 + idx, kb]
    dwpair = dwpair.reshape(256, 64).astype(ml_dtypes.bfloat16)

    # DVE tap columns per ct: w(0,1), w(2,1), w(2,2)
    dvecol = np.zeros((128, 15), np.float32)
    recip11 = np.zeros((128, 5), np.float32)
    for ct in range(5):
        idx = np.arange(CT[ct])
        dvecol[idx, 3 * ct + 0] = w9[CTO[ct] + idx, 1]
        dvecol[idx, 3 * ct + 1] = w9[CTO[ct] + idx, 7]
        dvecol[idx, 3 * ct + 2] = w9[CTO[ct] + idx, 8]
        recip11[idx, ct] = (1.0 / w11[CTO[ct] + idx]).astype(np.float32)

    temp = np.asarray(temperature).reshape(HEADS)
    tempcol = np.zeros((128, 2), np.float32)
    for h in range(HEADS):
        g, hl = divmod(h, 4)
        tempcol[24 * hl:24 * (hl + 1), g] = temp[h]

    in_maps = []
    for i in range(8):
        b, hf = divmod(i, 2)
        xin = np.zeros((C, HALF + 2, W), np.float32)
        r0 = hf * HALF - 1
        lo, hi = max(r0, 0), min(r0 + HALF + 2, H)
        xin[:, lo - r0:hi - r0, :] = x[b, :, lo:hi, :]
        in_maps.append({
            "xin": xin.reshape(C, (HALF + 2) * W).astype(ml_dtypes.bfloat16),
            "qkvwt": qkvwt, "projt": projt, "dwdiag": dwdiag,
            "dwpair": dwpair, "dvecol": dvecol, "recip11": recip11,
            "tempcol": tempcol,
        })
    return in_maps


def kernel(x, qkv_w, dw_w, proj_w, temperature):
    nc = _get_nc()
    in_maps = _shard_inputs(x, qkv_w, dw_w, proj_w, temperature)
    res = run_bass_kernel_spmd(nc, in_maps, core_ids=list(range(8)))
    out = np.empty((B, C, H, W), np.float32)
    for i in range(8):
        b, hf = divmod(i, 2)
        o = res.results[i]["out"].astype(np.float32).reshape(C, HALF, W)
        out[b, :, hf * HALF:(hf + 1) * HALF, :] = o
    return out
